# revision 14
# baseline (speedup 1.0000x reference)
"""PlasticNet (differentiable plasticity RNN) on 8 Trainium2 cores.

Batch-parallel: B=32 -> 4 samples/core. Each core runs the full T=64 scan
for its 4 samples with hebb [4,256,256] resident in SBUF.

Per-core layouts (fp32):
  xT   [128, 256]   xT[i, t*4+b] = x[t, 4c+b, i]
  w2   [128, 512]   w2[p, i0*256+k] = w[i0*128+p, k]
  alr  [128, 2048]  alr[p, i0*1024+b*256+k] = alpha[i0*128+p, k]  (b-replicated)
  hebb [128, 2048]  hebb[p, i0*1024+b*256+k] = hebb[b, i0*128+p, k]
  ys   [4, 16384]   ys[b, t*256+k] = h(t)[b,k]

Math per step t (h_pred = h(t-1), hebb = hebb(t-1)):
  rec[b,k] = sum_i h_pred[b,i]*(w[i,k] + alpha[i,k]*hebb[b,i,k])
  h = tanh(x_t @ Wi + bi + rec)
  eta = tanh(h @ Wm + bm);  g[b,k] = (eta[b]*Wf[k]+bf[k]) * h[b,k]
  hebb = clip(hebb + h_pred[b,i]*g[b,k], -2, 2)
"""

import numpy as np

T, B, I, H = 64, 32, 128, 256
CLIP = 2.0
NCORES = 8
BL = B // NCORES  # 4

_CACHE = {}


def _build():
    from contextlib import ExitStack
    from concourse import bass, bacc
    from concourse import tile
    from concourse.alu_op_type import AluOpType as Alu

    dt = bass.mybir.dt
    f32 = dt.float32
    Act = bass.mybir.ActivationFunctionType

    nc = bacc.Bacc("TRN2", target_bir_lowering=False, debug=False)

    xT_d = nc.dram_tensor("xT", [I, T * BL], f32, kind="ExternalInput").ap()
    wi_d = nc.dram_tensor("wi", [I, H], f32, kind="ExternalInput").ap()
    bi_d = nc.dram_tensor("bi1", [1, H], f32, kind="ExternalInput").ap()
    w2_d = nc.dram_tensor("w2", [I, 2 * H], f32, kind="ExternalInput").ap()
    al_d = nc.dram_tensor("alr", [I, 2 * BL * H], f32, kind="ExternalInput").ap()
    wm_d = nc.dram_tensor("wmr", [BL, H], f32, kind="ExternalInput").ap()
    bm_d = nc.dram_tensor("bmr", [BL, 1], f32, kind="ExternalInput").ap()
    wf_d = nc.dram_tensor("wfr", [BL, H], f32, kind="ExternalInput").ap()
    bf_d = nc.dram_tensor("bfr", [BL, H], f32, kind="ExternalInput").ap()
    se_d = nc.dram_tensor("sel", [BL, BL * BL], f32, kind="ExternalInput").ap()
    on_d = nc.dram_tensor("ones1", [1, BL], f32, kind="ExternalInput").ap()
    id_d = nc.dram_tensor("id4", [BL, BL], f32, kind="ExternalInput").ap()

    ys_d = nc.dram_tensor("ys", [BL, T * H], f32, kind="ExternalOutput").ap()
    hb_d = nc.dram_tensor("hebb", [I, 2 * BL * H], f32, kind="ExternalOutput").ap()

    with tile.TileContext(nc) as tc:
        with ExitStack() as ctx:
            sb = ctx.enter_context(tc.tile_pool(name="sb", bufs=1))
            pp = ctx.enter_context(tc.tile_pool(name="pp", bufs=1, space="PSUM"))

            x_sb = sb.tile([I, T * BL], f32)
            wi_sb = sb.tile([I, H], f32)
            bi_sb = sb.tile([1, H], f32)
            w2_sb = sb.tile([I, 2 * H], f32)
            al_sb = sb.tile([I, 2 * BL * H], f32)
            wm_sb = sb.tile([BL, H], f32)
            bm_sb = sb.tile([BL, 1], f32)
            wf_sb = sb.tile([BL, H], f32)
            bf_sb = sb.tile([BL, H], f32)
            se_sb = sb.tile([BL, BL * BL], f32)
            on_sb = sb.tile([1, BL], f32)
            id_sb = sb.tile([BL, BL], f32)

            ys_sb = sb.tile([BL, T * H], f32)
            hebb_sb = sb.tile([I, 2 * BL * H], f32)
            t1_sb = sb.tile([I, 2 * BL * H], f32)  # alpha * hebb
            z0 = sb.tile([I, 4 * BL], f32)  # hT chunk 0, spread cols {0,5,10,15}
            z1 = sb.tile([I, 4 * BL], f32)
            gbd_sb = sb.tile([BL, BL * H], f32)  # block-diag g
            eta_pre = sb.tile([BL, 1], f32)
            eta = sb.tile([BL, 1], f32)
            myeta = sb.tile([BL, H], f32)
            g = sb.tile([BL, H], f32)
            scr = sb.tile([BL, H], f32)

            # PSUM: alloc order keeps every matmul dest inside one 2KB bank
            de_ps = pp.tile([I, 2 * BL * H], f32)  # 8KB -> banks 0-3
            gbd_ps = pp.tile([BL, BL * H], f32)  # 4KB -> banks 4-5
            rec_ps = pp.tile([BL, H], f32)  # 1KB
            tp_ps = pp.tile([I, 2 * BL], f32)  # 32B

            for dst, src in [
                (x_sb, xT_d), (wi_sb, wi_d), (bi_sb, bi_d), (w2_sb, w2_d),
                (al_sb, al_d), (wm_sb, wm_d), (bm_sb, bm_d), (wf_sb, wf_d),
                (bf_sb, bf_d), (se_sb, se_d), (on_sb, on_d), (id_sb, id_d),
            ]:
                nc.sync.dma_start(dst[:], src)

            nc.vector.memset(hebb_sb[:], 0.0)
            nc.vector.memset(z0[:], 0.0)
            nc.vector.memset(z1[:], 0.0)

            for t in range(T):
                hs = ys_sb[:, t * H:(t + 1) * H]  # h(t) slot

                # ---- rec accumulation: x@Wi + bi (+ h_prev@w + h_prev@(alpha*hebb))
                nmm = 2 + (2 if t >= 1 else 0) + (8 if t >= 2 else 0)
                k = 1
                nc.tensor.matmul(rec_ps[:], x_sb[:, t * BL:(t + 1) * BL],
                                 wi_sb[:], start=True, stop=False)
                k += 1
                nc.tensor.matmul(rec_ps[:], on_sb[:], bi_sb[:],
                                 start=False, stop=(k == nmm))
                if t >= 1:
                    for i0, z in ((0, z0), (1, z1)):
                        k += 1
                        nc.tensor.matmul(rec_ps[:], z[:, 0:4 * BL:BL + 1],
                                         w2_sb[:, i0 * H:(i0 + 1) * H],
                                         start=False, stop=(k == nmm))
                if t >= 2:
                    for i0, z in ((0, z0), (1, z1)):
                        for b in range(BL):
                            k += 1
                            nc.tensor.matmul(
                                rec_ps[:], z[:, BL * b:BL * (b + 1)],
                                t1_sb[:, (i0 * BL + b) * H:(i0 * BL + b + 1) * H],
                                start=False, stop=(k == nmm))

                # ---- h = tanh(rec)
                nc.scalar.activation(hs, rec_ps[:], Act.Tanh)

                # ---- hT -> z (feeds next step's rec matmuls)
                if t < T - 1:
                    for i0, z in ((0, z0), (1, z1)):
                        nc.tensor.transpose(tp_ps[:, BL * i0:BL * (i0 + 1)],
                                            hs[:, i0 * I:(i0 + 1) * I], id_sb[:])
                        nc.vector.tensor_copy(z[:, 0:4 * BL:BL + 1],
                                              tp_ps[:, BL * i0:BL * (i0 + 1)])

                if t >= 1:
                    # ---- eta = tanh(h@Wm + bm); g = (eta*Wf + bf) * h
                    nc.vector.scalar_tensor_tensor(scr[:], hs, 1.0, wm_sb[:],
                                                   Alu.bypass, Alu.mult,
                                                   accum_out=eta_pre[:])
                    nc.scalar.activation(eta[:], eta_pre[:], Act.Tanh, bias=bm_sb[:])
                    nc.vector.scalar_tensor_tensor(myeta[:], wf_sb[:], eta[:],
                                                   bf_sb[:], Alu.mult, Alu.add)
                    nc.vector.tensor_tensor(g[:], myeta[:], hs, Alu.mult)

                    # ---- block-diag g: gbd[b, b*H+k] = g[b,k]
                    for b in range(BL):
                        nc.tensor.matmul(gbd_ps[:, b * H:(b + 1) * H],
                                         se_sb[:, BL * b:BL * (b + 1)], g[:],
                                         start=True, stop=True)
                    nc.scalar.activation(gbd_sb[:], gbd_ps[:], Act.Copy)

                    # ---- delta[b,i,k] = h_prev[b,i] * g[b,k]
                    hp = ys_sb[:, (t - 1) * H:t * H]
                    for i0 in range(2):
                        for half in range(2):
                            nc.tensor.matmul(
                                de_ps[:, i0 * BL * H + half * 2 * H:
                                      i0 * BL * H + (half + 1) * 2 * H],
                                hp[:, i0 * I:(i0 + 1) * I],
                                gbd_sb[:, half * 2 * H:(half + 1) * 2 * H],
                                start=True, stop=True)

                    # ---- hebb = clip(hebb + delta); t1 = alpha*hebb
                    nc.vector.tensor_tensor(hebb_sb[:], hebb_sb[:], de_ps[:],
                                            Alu.add)
                    nc.vector.tensor_scalar(hebb_sb[:], hebb_sb[:], -CLIP, CLIP,
                                            Alu.max, Alu.min)
                    if t < T - 1:
                        nc.vector.tensor_tensor(t1_sb[:], al_sb[:], hebb_sb[:],
                                                Alu.mult)

            nc.sync.dma_start(ys_d, ys_sb[:])
            nc.sync.dma_start(hb_d, hebb_sb[:])

    nc.compile()
    return nc


def _get_program():
    if "nc" not in _CACHE:
        _CACHE["nc"] = _build()
    return _CACHE["nc"]


def _make_in_maps(x, Wi, bi, w, alpha, Wm, bm, Wf, bf):
    f = np.float32
    c32 = lambda a: np.ascontiguousarray(a, dtype=f)

    sel = np.zeros((BL, BL * BL), dtype=f)
    for b in range(BL):
        sel[b, BL * b + b] = 1.0

    shared = {
        "wi": c32(Wi),
        "bi1": c32(bi.reshape(1, H)),
        "w2": c32(w.reshape(2, I, H).transpose(1, 0, 2).reshape(I, 2 * H)),
        "alr": c32(np.broadcast_to(
            alpha.reshape(2, I, H).transpose(1, 0, 2)[:, :, None, :],
            (I, 2, BL, H)).reshape(I, 2 * BL * H)),
        "wmr": c32(np.broadcast_to(Wm[:, 0][None, :], (BL, H))),
        "bmr": c32(np.full((BL, 1), bm[0])),
        "wfr": c32(np.broadcast_to(Wf, (BL, H))),
        "bfr": c32(np.broadcast_to(bf[None, :], (BL, H))),
        "sel": sel,
        "ones1": np.ones((1, BL), dtype=f),
        "id4": np.eye(BL, dtype=f),
    }

    in_maps = []
    for c in range(NCORES):
        m = dict(shared)
        m["xT"] = c32(x[:, BL * c:BL * (c + 1), :].transpose(2, 0, 1)
                      .reshape(I, T * BL))
        in_maps.append(m)
    return in_maps


def _gather(results):
    f = np.float32
    ys = np.empty((T, B, H), dtype=f)
    hebb = np.empty((B, H, H), dtype=f)
    for c in range(NCORES):
        out = results[c]
        ys[:, BL * c:BL * (c + 1), :] = (
            out["ys"].reshape(BL, T, H).transpose(1, 0, 2))
        hebb[BL * c:BL * (c + 1)] = (
            out["hebb"].reshape(I, 2, BL, H).transpose(2, 1, 0, 3)
            .reshape(BL, H, H))
    h_f = ys[T - 1].copy()
    return ys, h_f, hebb


def kernel(x, Wi, bi, w, alpha, Wm, bm, Wf, bf):
    from concourse.bass_utils import run_bass_kernel_spmd

    nc = _get_program()
    in_maps = _make_in_maps(x, Wi, bi, w, alpha, Wm, bm, Wf, bf)
    res = run_bass_kernel_spmd(nc, in_maps, list(range(NCORES)))
    return _gather(res.results)
